# revision 7
# baseline (speedup 1.0000x reference)
"""SchNet-style GNN (6 SchNetConv layers + output block) on 8 Trainium2 cores.

Strategy
--------
Edges are sorted by destination node; destination nodes are split into 8
contiguous shards with ~equal edge counts (graph partitioning per the
sharding hint). Each core owns one node shard and the edges that point into
it. Per layer:
  - every core holds the FULL node-feature table h (replicated via AllGather)
  - h[j] rows are fetched with one `dma_gather` per (chunk, index-window):
    indices are int16, so the 50k-row table is addressed through two 32768-row
    windows; edges within a chunk are ordered low-window-first (legal: the
    segment sum is order-free within a chunk)
  - edge MLP:  W1T[h,e] = fw1^T @ rbf^T  (PE), ssp via Exp+Ln (ACT, both
    functions pinned to the one natural_log_exp table set),
    W[e,h'] = A @ fw2 + fb2'  (PE, rank-1 bias matmul)
  - m = h[j] * W (DVE); segment-sum via selection-matrix matmul:
    S[e,n] = (dest_local == iota), aggr[n,h] += S^T @ m accumulated in PSUM
    over each 128-node chunk (dummy padding edges have dest_local=999 so
    their S row is zero)
  - node MLP in alternating orientations (one PE transpose per chunk),
    h_new -> AllGather input for the next layer.
The softplus-0.5 constants are folded into downstream biases on the host.
rbf is computed on-device in a prologue from host-prepared (1, d, d^2)
features via a block-diagonal quadratic matmul + Exp(scale=-gamma).
Output block runs on the core's own shard; per-graph partial sums are
produced with the same selection-matrix trick and combined on the host.
"""
import os
import sys

sys.path.insert(0, "/opt/trn_rl_repo")

import numpy as np

# ---- problem constants (hardcoded per harness contract) ----
N_NODES = 50000
N_EDGES = 800000
N_GRAPHS = 512
HID = 128
RBF_DIM = 32
N_LAYERS = int(os.environ.get("GNN_LAYERS", "6"))
CUTOFF = 4.0
GAMMA = 0.5 / (CUTOFF / RBF_DIM) ** 2  # 32.0
CENTERS = np.linspace(0.0, CUTOFF, RBF_DIM).astype(np.float32)
NC = 8
P = 128
SB = 4            # tiles (of 128 edges) per edge compute superblock
WIN = 32768       # int16 index window for dma_gather
PAD_DL = 999.0    # dest_local marker for padding edges
PAD_BL = 10000.0  # batch_local marker for padding nodes

LAST_EXEC_NS = None
LAST_RESULTS = None


def _ceil(a, b):
    return -(-a // b)


# --------------------------------------------------------------------------
# host-side preprocessing
# --------------------------------------------------------------------------
def _prep(z, pos, edge_index, batch, embed, fw1, fb1, fw2, fb2,
          uw1, ub1, uw2, ub2, ow1, ob1, ow2, ob2, ow3, ob3):
    f32 = np.float32
    z = np.asarray(z).astype(np.int64)
    pos = np.asarray(pos).astype(f32)
    ei = np.asarray(edge_index).astype(np.int64)
    batch = np.asarray(batch).astype(np.int64)
    i, j = ei[0], ei[1]

    deg = np.bincount(i, minlength=N_NODES).astype(f32)
    inv_deg = (1.0 / np.maximum(deg, 1.0)).astype(f32)

    order = np.argsort(i, kind="stable")
    i_s = i[order]
    j_s = j[order]
    rel = pos[i_s] - pos[j_s]
    d = np.sqrt((rel * rel).sum(axis=1)).astype(f32)

    # edge-count prefix over nodes; split nodes into 8 shards w/ ~equal edges
    epre = np.concatenate([[0], np.cumsum(deg.astype(np.int64))])
    nb = [0]
    for k in range(1, NC):
        nb.append(int(np.searchsorted(epre, k * N_EDGES // NC)))
    nb.append(N_NODES)
    nb = np.array(nb, dtype=np.int64)
    nk = nb[1:] - nb[:-1]
    S_pad = int(_ceil(int(nk.max()), P) * P)
    n_chunks = S_pad // P
    S_tot = NC * S_pad
    assert S_tot - WIN <= WIN, "high window must cover the rest"

    # owner + padded coordinate of each (sorted) edge's source node
    owner = np.searchsorted(nb[1:], j_s, side="right")
    j_pad = (owner * S_pad + (j_s - nb[owner])).astype(np.int64)
    is_hi = j_pad >= WIN

    # per (core, chunk) edge ranges; per-window tile counts uniform over cores
    e_lo = np.zeros((NC, n_chunks), dtype=np.int64)
    e_hi = np.zeros((NC, n_chunks), dtype=np.int64)
    cnt_lo = np.zeros((NC, n_chunks), dtype=np.int64)
    for k in range(NC):
        for c in range(n_chunks):
            lo_node = min(nb[k] + P * c, nb[k + 1])
            hi_node = min(nb[k] + P * (c + 1), nb[k + 1])
            e_lo[k, c] = epre[lo_node]
            e_hi[k, c] = epre[hi_node]
            cnt_lo[k, c] = int((~is_hi[epre[lo_node]:epre[hi_node]]).sum())
    cnt = e_hi - e_lo
    cnt_hi = cnt - cnt_lo
    T_lo = _ceil(cnt_lo, P).max(axis=0).astype(np.int64)
    T_hi = _ceil(cnt_hi, P).max(axis=0).astype(np.int64)
    T_lo = np.maximum(T_lo, 1)  # ensure every chunk has >=1 tile
    # pad total tiles to a multiple of 16 (prologue groups of 2048 edges)
    TT = int((T_lo + T_hi).sum())
    T_lo[-1] += (-TT) % 16
    T_c = T_lo + T_hi
    TT = int(T_c.sum())
    tile_base = np.concatenate([[0], np.cumsum(T_c)])

    tile_chunk = np.zeros(TT, dtype=np.int64)
    for c in range(n_chunks):
        tile_chunk[tile_base[c]:tile_base[c + 1]] = c

    # folded biases (ssp(x) = softplus(x) - 0.5; the -0.5 folds downstream)
    fw1 = np.asarray(fw1).astype(f32); fb1 = np.asarray(fb1).astype(f32)
    fw2 = np.asarray(fw2).astype(f32); fb2 = np.asarray(fb2).astype(f32)
    uw1 = np.asarray(uw1).astype(f32); ub1 = np.asarray(ub1).astype(f32)
    uw2 = np.asarray(uw2).astype(f32); ub2 = np.asarray(ub2).astype(f32)
    ow1 = np.asarray(ow1).astype(f32); ob1 = np.asarray(ob1).astype(f32)
    ow2 = np.asarray(ow2).astype(f32); ob2 = np.asarray(ob2).astype(f32)
    ow3 = np.asarray(ow3).astype(f32); ob3 = np.asarray(ob3).astype(f32)
    embed = np.asarray(embed).astype(f32)
    fb2p = fb2 - 0.5 * fw2.sum(axis=1)          # [L, HID]
    ub2p = ub2 - 0.5 * uw2.sum(axis=1)          # [L, HID]
    ob2p = ob2 - 0.5 * ow2.sum(axis=0)          # [HID]
    ob3p = float(ob3[0] - 0.5 * ow3.sum())

    L = N_LAYERS
    bias_cols = np.zeros((P, 16), dtype=f32)
    for l in range(L):
        bias_cols[:, l] = fb1[l]
        bias_cols[:, 6 + l] = ub1[l]
    bias_cols[:, 12] = ob1
    bias_cols[:, 13] = ob2p
    # rank-1 operands live on partition 0 as column segments (PE requires
    # base_partition 0 for both matmul operands)
    rank1_128 = np.zeros((1, 16 * P), dtype=f32)
    rank1_128[0, 0:P] = 1.0                      # ones segment
    for l in range(L):
        rank1_128[0, (7 + l) * P:(8 + l) * P] = ub2p[l]
    rank1_128[0, 13 * P:14 * P] = ub2p[L - 1]    # last-layer (h6T) bias
    rank1_512 = np.zeros((1, 8 * 4 * P), dtype=f32)
    for l in range(L):
        rank1_512[0, l * 4 * P:(l + 1) * 4 * P] = np.tile(fb2p[l], 4)

    iota = np.broadcast_to(np.arange(P, dtype=f32)[None, :], (P, P)).copy()

    qmat4 = np.zeros((12, P), dtype=f32)
    for b in range(4):
        qmat4[3 * b + 0, 32 * b:32 * b + 32] = CENTERS * CENTERS
        qmat4[3 * b + 1, 32 * b:32 * b + 32] = -2.0 * CENTERS
        qmat4[3 * b + 2, 32 * b:32 * b + 32] = 1.0

    meta = dict(S_pad=S_pad, n_chunks=n_chunks, TT=TT, L=L,
                T_lo=T_lo, T_hi=T_hi, T_c=T_c, tile_base=tile_base,
                tile_chunk=tile_chunk)

    # ---- per-core arrays ----
    in_maps = []
    gb = np.zeros((NC, n_chunks), dtype=np.int64)
    jdbg = []
    for k in range(NC):
        jarr = np.zeros(TT * P, dtype=np.int64)      # padded global coords
        j16 = np.zeros(TT * P, dtype=np.int16)       # window-local indices
        dlarr = np.full(TT * P, PAD_DL, dtype=f32)
        darr = np.zeros(TT * P, dtype=f32)
        onearr = np.zeros(TT * P, dtype=f32)
        for c in range(n_chunks):
            lo, hi = e_lo[k, c], e_hi[k, c]
            hi_mask = is_hi[lo:hi]
            # low-window edges first, then high-window
            ordr = np.concatenate([np.nonzero(~hi_mask)[0],
                                   np.nonzero(hi_mask)[0]]) + lo
            nlo = int((~hi_mask).sum())
            jp = j_pad[ordr]
            dl = (i_s[ordr] - (nb[k] + P * c)).astype(f32)
            dd = d[ordr]
            base_lo = tile_base[c] * P
            base_hi = (tile_base[c] + T_lo[c]) * P
            # low run
            jarr[base_lo:base_lo + nlo] = jp[:nlo]
            j16[base_lo:base_lo + nlo] = jp[:nlo].astype(np.int16)
            dlarr[base_lo:base_lo + nlo] = dl[:nlo]
            darr[base_lo:base_lo + nlo] = dd[:nlo]
            onearr[base_lo:base_lo + nlo] = 1.0
            # high run
            nhi = len(ordr) - nlo
            jarr[base_hi:base_hi + nhi] = jp[nlo:]
            j16[base_hi:base_hi + nhi] = (jp[nlo:] - WIN).astype(np.int16)
            dlarr[base_hi:base_hi + nhi] = dl[nlo:]
            darr[base_hi:base_hi + nhi] = dd[nlo:]
            onearr[base_hi:base_hi + nhi] = 1.0

        # per-run int16 index blocks in dma_gather wrap order:
        # block [16, T_run*8] = run.reshape(T_run*8, 16).T
        j16w = np.zeros((16, TT * 8), dtype=np.int16)
        for c in range(n_chunks):
            for (tb, tr) in ((tile_base[c], T_lo[c]),
                             (tile_base[c] + T_lo[c], T_hi[c])):
                if tr == 0:
                    continue
                run = j16[tb * P:(tb + tr) * P]
                j16w[:, tb * 8:(tb + tr) * 8] = run.reshape(tr * 8, 16).T

        # resident dest-local table [128, TT]
        dl_all = np.ascontiguousarray(dlarr.reshape(TT, P).T)

        # distance features for the rbf prologue, block-diag groups of 2048
        NG = TT // 16
        df4 = np.zeros((NG, 12, 512), dtype=f32)
        feats = np.stack([onearr, darr, darr * darr])  # [3, TT*128]
        for b in range(4):
            df4[:, 3 * b:3 * b + 3, :] = (
                feats.reshape(3, NG, 4, 512)[:, :, b, :].transpose(1, 0, 2))

        # node-side arrays
        nkk = int(nk[k])
        ivd = np.ones(S_pad, dtype=f32)
        ivd[:nkk] = inv_deg[nb[k]:nb[k + 1]]
        ivd_sw = np.ascontiguousarray(ivd.reshape(n_chunks, P).T)
        bl = np.full(S_pad, PAD_BL, dtype=f32)
        for c in range(n_chunks):
            lo_node = nb[k] + P * c
            if lo_node < nb[k + 1]:
                g0 = int(batch[lo_node])
                gb[k, c] = g0
                hi_node = min(lo_node + P, nb[k + 1])
                bl[P * c: P * c + hi_node - lo_node] = (
                    batch[lo_node:hi_node] - g0).astype(f32)
        bl_sw = np.ascontiguousarray(bl.reshape(n_chunks, P).T)

        zsh = np.zeros(S_pad, dtype=np.int64)
        zsh[:nkk] = z[nb[k]:nb[k + 1]]
        h0 = embed[zsh].astype(f32)

        jdbg.append(jarr.copy())
        in_maps.append({
            "h0": h0, "df4": df4, "j16w": j16w, "dl_all": dl_all,
            "ivd": ivd_sw, "blsw": bl_sw, "iota": iota, "qmat": qmat4,
            "bias_cols": bias_cols, "rank1a": rank1_128, "rank1b": rank1_512,
            "fw1s": fw1[:L], "fw2s": fw2[:L], "uw1s": uw1[:L], "uw2s": uw2[:L],
            "ow1": ow1, "ow2": ow2, "ow3": ow3,
        })

    post = dict(gb=gb, ob3p=ob3p, batch=np.asarray(batch), jdbg=jdbg)
    return meta, in_maps, post


def _patch_act_tables():
    """Make bass's table-load pass map both Exp and Ln onto the combined
    natural_log_exp_and_others set (otherwise it alternates between two
    sets and pays a ~1.3us table load per activation). Only the *choice*
    input is doctored; the runtime tables for the chosen set genuinely
    contain both functions."""
    from concourse import hw_specs, bacc, mybir
    AF = mybir.ActivationFunctionType
    if getattr(bacc, "_gnn_act_patch", False):
        return
    orig = hw_specs.get_activation_tables

    def patched(arch):
        t = {k: set(v) for k, v in orig(arch).items()}
        for name in t:
            if name != "natural_log_exp_and_others":
                t[name] = t[name] - {AF.Exp, AF.Ln}
        return t

    bacc.get_activation_tables = patched
    bacc._gnn_act_patch = True


# --------------------------------------------------------------------------
# device kernel builder (single SPMD program for all 8 cores)
# --------------------------------------------------------------------------
def _build(meta, repeat=1):
    import concourse.tile as tile
    from concourse import bacc, bass, mybir
    from concourse.masks import make_identity

    _patch_act_tables()

    f32 = mybir.dt.float32
    i16 = mybir.dt.int16
    AF = mybir.ActivationFunctionType
    S_pad = meta["S_pad"]; n_chunks = meta["n_chunks"]
    TT = meta["TT"]; L = meta["L"]
    T_lo = meta["T_lo"]; T_hi = meta["T_hi"]; T_c = meta["T_c"]
    tile_base = meta["tile_base"]
    NG = TT // 16
    S_tot = NC * S_pad
    T_max = int(T_c.max())

    nc = bacc.Bacc("TRN2", target_bir_lowering=False, debug=False,
                   num_devices=NC)

    h0 = nc.dram_tensor("h0", [S_pad, HID], f32, kind="ExternalInput")
    df4 = nc.dram_tensor("df4", [NG, 12, 512], f32, kind="ExternalInput")
    j16wi = nc.dram_tensor("j16w", [16, TT * 8], i16, kind="ExternalInput")
    dl_alli = nc.dram_tensor("dl_all", [P, TT], f32, kind="ExternalInput")
    ivd = nc.dram_tensor("ivd", [P, n_chunks], f32, kind="ExternalInput")
    blsw = nc.dram_tensor("blsw", [P, n_chunks], f32, kind="ExternalInput")
    iota = nc.dram_tensor("iota", [P, P], f32, kind="ExternalInput")
    qmat = nc.dram_tensor("qmat", [12, P], f32, kind="ExternalInput")
    bias_cols = nc.dram_tensor("bias_cols", [P, 16], f32, kind="ExternalInput")
    rank1a = nc.dram_tensor("rank1a", [1, 16 * P], f32, kind="ExternalInput")
    rank1b = nc.dram_tensor("rank1b", [1, 8 * 4 * P], f32,
                            kind="ExternalInput")
    fw1s = nc.dram_tensor("fw1s", [L, RBF_DIM, HID], f32, kind="ExternalInput")
    fw2s = nc.dram_tensor("fw2s", [L, HID, HID], f32, kind="ExternalInput")
    uw1s = nc.dram_tensor("uw1s", [L, HID, HID], f32, kind="ExternalInput")
    uw2s = nc.dram_tensor("uw2s", [L, HID, HID], f32, kind="ExternalInput")
    ow1 = nc.dram_tensor("ow1", [HID, HID], f32, kind="ExternalInput")
    ow2 = nc.dram_tensor("ow2", [HID, HID], f32, kind="ExternalInput")
    ow3 = nc.dram_tensor("ow3", [HID, 1], f32, kind="ExternalInput")
    epart = nc.dram_tensor("epart", [P, n_chunks], f32, kind="ExternalOutput")

    rg = [list(range(NC))]

    with tile.TileContext(nc) as tc:
        from contextlib import ExitStack
        with ExitStack() as ctx:
            sb = ctx.enter_context(tc.tile_pool(name="sbuf", bufs=3))
            sbc = ctx.enter_context(tc.tile_pool(name="const", bufs=1))
            ps = ctx.enter_context(tc.tile_pool(name="psum", bufs=1,
                                                space="PSUM"))
            dr = ctx.enter_context(tc.tile_pool(name="dram", bufs=1,
                                                space="DRAM"))

            # ---- resident constants ----
            iota_t = sbc.tile([P, P], f32)
            nc.sync.dma_start(out=iota_t[:], in_=iota[:])
            ivd_t = sbc.tile([P, n_chunks], f32)
            nc.sync.dma_start(out=ivd_t[:], in_=ivd[:])
            bl_t = sbc.tile([P, n_chunks], f32)
            nc.sync.dma_start(out=bl_t[:], in_=blsw[:])
            bias_t = sbc.tile([P, 16], f32)
            nc.sync.dma_start(out=bias_t[:], in_=bias_cols[:])
            r1a_t = sbc.tile([1, 16 * P], f32)
            nc.sync.dma_start(out=r1a_t[:], in_=rank1a[:])
            r1b_t = sbc.tile([1, 8 * 4 * P], f32)
            nc.sync.dma_start(out=r1b_t[:], in_=rank1b[:])
            qm_t = sbc.tile([12, P], f32)
            nc.sync.dma_start(out=qm_t[:], in_=qmat[:])
            ident = sbc.tile([P, P], f32)
            make_identity(nc, ident[:])
            ow1_t = sbc.tile([HID, HID], f32)
            nc.sync.dma_start(out=ow1_t[:], in_=ow1[:])
            ow2_t = sbc.tile([HID, HID], f32)
            nc.sync.dma_start(out=ow2_t[:], in_=ow2[:])
            ow3_t = sbc.tile([HID, 1], f32)
            nc.sync.dma_start(out=ow3_t[:], in_=ow3[:])
            # gather indices: [16, TT*8] block replicated across the 8
            # gpsimd-core partition groups
            j16_t = sbc.tile([P, TT * 8], i16)
            for g in range(8):
                nc.sync.dma_start(out=j16_t[16 * g:16 * (g + 1), :],
                                  in_=j16wi[:])
            dl_t = sbc.tile([P, TT], f32)
            nc.sync.dma_start(out=dl_t[:], in_=dl_alli[:])
            h6T = sbc.tile([P, n_chunks * P], f32)
            epart_t = sbc.tile([P, n_chunks], f32)

            for rep in range(repeat):
                # ---- DRAM scratch ----
                rbfT = dr.tile([RBF_DIM, TT * P], f32, name=f"rbfT{rep}")
                agin = [dr.tile([S_pad, HID], f32, name=f"agin{rep}_{x}")
                        for x in range(2)]
                hfull = [dr.tile([S_tot, HID], f32, addr_space="Shared",
                                 name=f"hfull{rep}_{x}") for x in range(L)]

                # ---- prologue: rbf^T from quadratic features ----
                for g in range(NG):
                    dft = sb.tile([12, 512], f32, tag="dft")
                    nc.sync.dma_start(out=dft[:], in_=df4[g])
                    q_ps = ps.tile([P, 512], f32, tag="w1t", bufs=2)
                    nc.tensor.matmul(q_ps[:], lhsT=qm_t[:], rhs=dft[:],
                                     start=True, stop=True)
                    rb_sb = sb.tile([P, 512], f32, tag="rbf")
                    nc.scalar.activation(rb_sb[:], q_ps[:], AF.Exp,
                                         bias=0.0, scale=-float(GAMMA))
                    for b in range(4):
                        col = g * 2048 + b * 512
                        nc.sync.dma_start(
                            out=rbfT[:, col:col + 512],
                            in_=rb_sb[32 * b:32 * b + 32, :])

                # ---- h0 -> AllGather -> hfull[0] ----
                nc.gpsimd.dma_start(out=agin[0][:], in_=h0[:])
                nc.gpsimd.collective_compute(
                    "AllGather", mybir.AluOpType.bypass, replica_groups=rg,
                    ins=[agin[0].opt()], outs=[hfull[0].opt()])

                # ---- layers ----
                for l in range(L):
                    hin = hfull[l]
                    fw1_t = sb.tile([RBF_DIM, HID], f32, tag="wfw1", bufs=2)
                    nc.sync.dma_start(out=fw1_t[:], in_=fw1s[l])
                    fw2_t = sb.tile([HID, HID], f32, tag="wfw2", bufs=2)
                    nc.sync.dma_start(out=fw2_t[:], in_=fw2s[l])
                    uw1_t = sb.tile([HID, HID], f32, tag="wuw1", bufs=2)
                    nc.sync.dma_start(out=uw1_t[:], in_=uw1s[l])
                    uw2_t = sb.tile([HID, HID], f32, tag="wuw2", bufs=2)
                    nc.sync.dma_start(out=uw2_t[:], in_=uw2s[l])

                    aggT4 = None
                    nb_first_chunk = 0

                    for c in range(n_chunks):
                        tb = int(tile_base[c])
                        tlo, thi = int(T_lo[c]), int(T_hi[c])
                        tcc = tlo + thi
                        # gather all h[j] rows for this chunk (1-2 ops)
                        hjc = sb.tile([P, T_max * P], f32, tag="hj", bufs=2)
                        if tlo:
                            nc.gpsimd.dma_gather(
                                out_ap=hjc[:, :tlo * P].rearrange(
                                    "p (a b) -> p a b", b=P),
                                in_ap=hin[0:WIN, :],
                                idxs_ap=j16_t[:16, tb * 8:(tb + tlo) * 8],
                                num_idxs=tlo * P, num_idxs_reg=tlo * P,
                                elem_size=HID, single_packet=False)
                        if thi:
                            nc.gpsimd.dma_gather(
                                out_ap=hjc[:, tlo * P:tcc * P].rearrange(
                                    "p (a b) -> p a b", b=P),
                                in_ap=hin[WIN:S_tot, :],
                                idxs_ap=j16_t[:16,
                                              (tb + tlo) * 8:(tb + tcc) * 8],
                                num_idxs=thi * P, num_idxs_reg=thi * P,
                                elem_size=HID, single_packet=False)

                        aggr_ps = ps.tile([P, HID], f32, tag="aggr", bufs=2)
                        for t0 in range(0, tcc, SB):
                            w = min(SB, tcc - t0)
                            gt0 = tb + t0
                            wcols = slice(t0 * P, (t0 + w) * P)
                            rbt = sb.tile([RBF_DIM, SB * P], f32, tag="rbt")
                            nc.sync.dma_start(
                                out=rbt[:, :w * P],
                                in_=rbfT[:, gt0 * P:(gt0 + w) * P])
                            w1_ps = ps.tile([P, SB * P], f32, tag="w1t",
                                            bufs=2)
                            nc.tensor.matmul(w1_ps[:, :w * P], lhsT=fw1_t[:],
                                             rhs=rbt[:, :w * P],
                                             start=True, stop=True)
                            ex_sb = sb.tile([P, SB * P], f32, tag="ex")
                            nc.scalar.activation(
                                ex_sb[:, :w * P], w1_ps[:, :w * P], AF.Exp,
                                bias=bias_t[:, l:l + 1], scale=1.0)
                            at_sb = sb.tile([P, SB * P], f32, tag="at")
                            nc.scalar.activation(
                                at_sb[:, :w * P], ex_sb[:, :w * P], AF.Ln,
                                bias=1.0, scale=1.0)

                            w_ps = ps.tile([P, SB * P], f32, tag="w", bufs=2)
                            for u in range(w):
                                ucols = slice(u * P, (u + 1) * P)
                                nc.tensor.matmul(
                                    w_ps[:, u * P:(u + 1) * P],
                                    lhsT=at_sb[:, ucols], rhs=fw2_t[:],
                                    start=True, stop=False)
                            nc.tensor.matmul(
                                w_ps[:, :w * P], lhsT=r1a_t[0:1, 0:P],
                                rhs=r1b_t[0:1, l * 4 * P:l * 4 * P + w * P],
                                start=False, stop=True)
                            m_sb = sb.tile([P, SB * P], f32, tag="m")
                            nc.vector.tensor_tensor(
                                out=m_sb[:, :w * P], in0=hjc[:, wcols],
                                in1=w_ps[:, :w * P],
                                op=mybir.AluOpType.mult)
                            s_sb = sb.tile([P, SB * P], f32, tag="s")
                            nc.vector.tensor_tensor(
                                out=s_sb[:, :w * P].rearrange(
                                    "p (a b) -> p a b", b=P),
                                in0=dl_t[:, gt0:gt0 + w].to_broadcast(
                                    [P, w, P]),
                                in1=iota_t[:].rearrange(
                                    "p (a b) -> p a b", a=1).to_broadcast(
                                    [P, w, P]),
                                op=mybir.AluOpType.is_equal)
                            for u in range(w):
                                ucols = slice(u * P, (u + 1) * P)
                                nc.tensor.matmul(
                                    aggr_ps[:], lhsT=s_sb[:, ucols],
                                    rhs=m_sb[:, ucols],
                                    start=(t0 + u == 0),
                                    stop=(t0 + u == tcc - 1))

                        # ---- chunk complete: node-side processing ----
                        aggs = sb.tile([P, HID], f32, tag="aggs")
                        nc.vector.tensor_scalar_mul(
                            aggs[:], aggr_ps[:], ivd_t[:, c:c + 1])
                        tr_ps = ps.tile([P, P], f32, tag="tr")
                        nc.tensor.transpose(tr_ps[:], aggs[:], ident[:])
                        if c % 4 == 0:
                            aggT4 = sb.tile([P, 4 * P], f32, tag="agT")
                            nb_first_chunk = c
                        q = c - nb_first_chunk
                        nc.vector.tensor_copy(
                            out=aggT4[:, q * P:(q + 1) * P], in_=tr_ps[:])
                        if not (c % 4 == 3 or c == n_chunks - 1):
                            continue

                        # node MLP for chunks nb_first_chunk..c
                        width = (q + 1) * P
                        mid_ps = ps.tile([P, 4 * P], f32, tag="mid")
                        nc.tensor.matmul(mid_ps[:, :width], lhsT=uw1_t[:],
                                         rhs=aggT4[:, :width],
                                         start=True, stop=True)
                        mex = sb.tile([P, 4 * P], f32, tag="mex")
                        nc.scalar.activation(
                            mex[:, :width], mid_ps[:, :width], AF.Exp,
                            bias=bias_t[:, 6 + l:7 + l], scale=1.0)
                        mln = sb.tile([P, 4 * P], f32, tag="mln")
                        nc.scalar.activation(mln[:, :width], mex[:, :width],
                                             AF.Ln, bias=1.0, scale=1.0)
                        for q2 in range(q + 1):
                            cc = nb_first_chunk + q2
                            colq = slice(q2 * P, (q2 + 1) * P)
                            if l < L - 1:
                                hn_ps = ps.tile([P, HID], f32, tag="tr")
                                nc.tensor.matmul(hn_ps[:], lhsT=mln[:, colq],
                                                 rhs=uw2_t[:],
                                                 start=True, stop=False)
                                nc.tensor.matmul(
                                    hn_ps[:], lhsT=r1a_t[0:1, 0:P],
                                    rhs=r1a_t[0:1, (7 + l) * P:(8 + l) * P],
                                    start=False, stop=True)
                                hn_sb = sb.tile([P, HID], f32, tag="hn")
                                nc.vector.tensor_copy(out=hn_sb[:],
                                                      in_=hn_ps[:])
                                nc.sync.dma_start(
                                    out=agin[(l + 1) % 2][
                                        cc * P:(cc + 1) * P, :],
                                    in_=hn_sb[:])
                            else:
                                h6_ps = ps.tile([P, P], f32, tag="tr")
                                nc.tensor.matmul(h6_ps[:], lhsT=uw2_t[:],
                                                 rhs=mln[:, colq],
                                                 start=True, stop=False)
                                nc.tensor.matmul(
                                    h6_ps[:],
                                    lhsT=r1a_t[0:1, 13 * P:14 * P],
                                    rhs=r1a_t[0:1, 0:P],
                                    start=False, stop=True)
                                nc.vector.tensor_copy(
                                    out=h6T[:, cc * P:(cc + 1) * P],
                                    in_=h6_ps[:])

                    if l < L - 1:
                        nc.gpsimd.collective_compute(
                            "AllGather", mybir.AluOpType.bypass,
                            replica_groups=rg,
                            ins=[agin[(l + 1) % 2].opt()],
                            outs=[hfull[l + 1].opt()])

                # ---- output block over the local shard ----
                for ob in range(_ceil(n_chunks, 4)):
                    c0 = ob * 4
                    ncb = min(4, n_chunks - c0)
                    width = ncb * P
                    cols = slice(c0 * P, c0 * P + width)
                    e1_ps = ps.tile([P, 4 * P], f32, tag="mid")
                    nc.tensor.matmul(e1_ps[:, :width], lhsT=ow1_t[:],
                                     rhs=h6T[:, cols], start=True, stop=True)
                    s1x = sb.tile([P, 4 * P], f32, tag="mex")
                    nc.scalar.activation(s1x[:, :width], e1_ps[:, :width],
                                         AF.Exp, bias=bias_t[:, 12:13],
                                         scale=1.0)
                    s1 = sb.tile([P, 4 * P], f32, tag="mln")
                    nc.scalar.activation(s1[:, :width], s1x[:, :width],
                                         AF.Ln, bias=1.0, scale=1.0)
                    e2_ps = ps.tile([P, 4 * P], f32, tag="mid")
                    nc.tensor.matmul(e2_ps[:, :width], lhsT=ow2_t[:],
                                     rhs=s1[:, :width], start=True, stop=True)
                    s2x = sb.tile([P, 4 * P], f32, tag="mex")
                    nc.scalar.activation(s2x[:, :width], e2_ps[:, :width],
                                         AF.Exp, bias=bias_t[:, 13:14],
                                         scale=1.0)
                    s2 = sb.tile([P, 4 * P], f32, tag="mln")
                    nc.scalar.activation(s2[:, :width], s2x[:, :width],
                                         AF.Ln, bias=1.0, scale=1.0)
                    for q2 in range(ncb):
                        cc = c0 + q2
                        ea_ps = ps.tile([P, 1], f32, tag="tr")
                        nc.tensor.matmul(ea_ps[:],
                                         lhsT=s2[:, q2 * P:(q2 + 1) * P],
                                         rhs=ow3_t[:], start=True, stop=True)
                        ea_sb = sb.tile([P, 1], f32, tag="ea")
                        nc.vector.tensor_copy(out=ea_sb[:], in_=ea_ps[:])
                        sg = sb.tile([P, P], f32, tag="s")
                        nc.vector.tensor_tensor(
                            out=sg[:],
                            in0=bl_t[:, cc:cc + 1].to_broadcast([P, P]),
                            in1=iota_t[:], op=mybir.AluOpType.is_equal)
                        gp_ps = ps.tile([P, 1], f32, tag="aggr", bufs=2)
                        nc.tensor.matmul(gp_ps[:], lhsT=sg[:], rhs=ea_sb[:],
                                         start=True, stop=True)
                        nc.vector.tensor_copy(out=epart_t[:, cc:cc + 1],
                                              in_=gp_ps[:])
                nc.sync.dma_start(out=epart[:], in_=epart_t[:])

    nc.compile()
    return nc


# --------------------------------------------------------------------------
# entry point
# --------------------------------------------------------------------------
def kernel(**inputs):
    global LAST_EXEC_NS, LAST_RESULTS
    from concourse.bass_utils import run_bass_kernel_spmd

    meta, in_maps, post = _prep(**inputs)
    nc = _build(meta)
    res = run_bass_kernel_spmd(nc, in_maps, list(range(NC)))
    LAST_EXEC_NS = res.exec_time_ns
    LAST_RESULTS = res

    energy = np.zeros(N_GRAPHS, dtype=np.float64)
    gb = post["gb"]
    for k in range(NC):
        ep = res.results[k]["epart"]  # [128, n_chunks]
        for c in range(meta["n_chunks"]):
            g0 = int(gb[k, c])
            hi = min(N_GRAPHS, g0 + P)
            energy[g0:hi] += ep[:hi - g0, c].astype(np.float64)
    energy += post["ob3p"] * np.bincount(post["batch"], minlength=N_GRAPHS)
    return energy.astype(np.float32)


# --------------------------------------------------------------------------
# timing harness (test.py): jit once, pre-stage inputs, time repeat execs
# --------------------------------------------------------------------------
def make_timed_runner(nc, in_maps):
    import jax
    import numpy as _np
    from jax.sharding import Mesh, PartitionSpec, NamedSharding
    from jax.experimental.shard_map import shard_map
    from concourse import bass2jax, mybir
    from concourse.bass2jax import _bass_exec_p, partition_id_tensor

    bass2jax.install_neuronx_cc_hook()
    partition_name = (nc.partition_id_tensor.name
                      if nc.partition_id_tensor else None)
    in_names, out_names, out_avals, zero_outs = [], [], [], []
    for alloc in nc.m.functions[0].allocations:
        if not isinstance(alloc, mybir.MemoryLocationSet):
            continue
        name = alloc.memorylocations[0].name
        if alloc.kind == "ExternalInput":
            if name != partition_name:
                in_names.append(name)
        elif alloc.kind == "ExternalOutput":
            shape = tuple(alloc.tensor_shape)
            dtype = mybir.dt.np(alloc.dtype)
            out_names.append(name)
            out_avals.append(jax.core.ShapedArray(shape, dtype))
            zero_outs.append(_np.zeros(shape, dtype))
    n_params = len(in_names)
    all_in_names = list(in_names) + list(out_names)
    if partition_name is not None:
        all_in_names.append(partition_name)

    def _body(*args):
        operands = list(args)
        if partition_name is not None:
            operands.append(partition_id_tensor())
        return tuple(_bass_exec_p.bind(
            *operands, out_avals=tuple(out_avals),
            in_names=tuple(all_in_names), out_names=tuple(out_names),
            lowering_input_output_aliases=(),
            sim_require_finite=True, sim_require_nnan=True, nc=nc))

    devices = jax.devices()[:NC]
    mesh = Mesh(_np.asarray(devices), ("core",))
    spec = PartitionSpec("core")
    n_outs = len(out_names)
    sharded = jax.jit(
        shard_map(_body, mesh=mesh, in_specs=(spec,) * (n_params + n_outs),
                  out_specs=(spec,) * n_outs, check_rep=False),
        keep_unused=True)
    sh = NamedSharding(mesh, spec)
    staged = [jax.device_put(
        _np.concatenate([_np.asarray(in_maps[c][nm]) for c in range(NC)],
                        axis=0), sh) for nm in in_names]
    staged_zeros = [jax.device_put(
        _np.zeros((NC * z.shape[0], *z.shape[1:]), z.dtype), sh)
        for z in zero_outs]

    def run():
        outs = sharded(*staged, *staged_zeros)
        jax.block_until_ready(outs)
        return outs

    return run
